# revision 6
# baseline (speedup 1.0000x reference)
"""GAT encoder (3 GAT layers: 256-hid 4-head concat + mu/logvar 128) on 8 trn2 cores.

Strategy (dst-range node sharding, per sharding_hint):
 - Host sorts edges by dst, buckets per core (2500 dst nodes each), pads each
   128-node block to TPB tiles of 128 edges.
 - Phase A (per core, redundant): xp = x @ [W1 | Wss1 | Wsd1] for all nodes
   (fused weight cols give per-node src/dst logits), written to DRAM table XPT.
 - L1 edge phase: dma_gather xp rows by src (+per-node src logit), dma_gather
   dst logits; softmax coefs -> 4 scaled one-hot matrices per 128-edge tile,
   PSUM-accumulated staircase matmuls aggregate messages; denominators via a
   1/w rhs column.
 - L1 finalize per block: normalize, +bias, ELU -> h; PE-transpose h and
   matmul with [Wmu|vmu|umu]/[Wlv|vlv|ulv] to get next-layer message rows
   xpmu/xplv + logits; rows go into an AllGather across the 8 cores.
 - L2/3 edge phase: same staircase trick, H=1, both mu and lv from one gather.
Outputs (mu, logvar) assembled host-side from per-core slices.
"""

import numpy as np

# ---- problem constants (hardcoded per contract) ----
N = 20000
E = 320000
FIN = 512
HID = 256
LAT = 128
H = 4
C1 = 64
NEG = 0.2
EPS = 1e-16

NC = 8
NOWN = 2500          # dst nodes per core
BLOCKS = 20          # 128-node blocks per core (2560 padded local nodes)
NLOC = BLOCKS * 128  # 2560
TPB = 18             # tiles (128 edges) per block, fixed across cores
TILES = BLOCKS * TPB       # 360
EPAD = TILES * 128         # 46080 edge slots per core
TPC = 8                    # tiles per gather chunk
CHUNK = TPC * 128          # 1024 idxs per dma_gather (hw limit ~1024)
NCHUNK = TILES // TPC      # 45
NPADA = 157 * 128          # 20096 padded global rows for phase A
XW = 320                   # XPT row f32 width (xp 0:256, ss1 256:260)
X2W = 320                  # XPT2 row width (xpmu 0:128, xplv 128:256, ssmu 256, sslv 257)
EXW_ = 64                  # EXTD row width (sd1 0:4, sdmu 4, sdlv 5)

_cache = {}


def _wrap_idxs(idx):
    n = idx.shape[0]
    t = np.zeros((128, n // 16), np.int16)
    w = idx.reshape(n // 16, 16).T.astype(np.int16)
    for g in range(8):
        t[g * 16:(g + 1) * 16, :] = w
    return t


def _colmajor(a):
    # per-edge array [EPAD] -> [128, TILES] tile-column layout
    return np.ascontiguousarray(a.reshape(TILES, 128).T)


def _build_module():
    import concourse.bacc as bacc
    import concourse.mybir as mybir
    import concourse.tile as tile

    f32 = mybir.dt.float32
    i16 = mybir.dt.int16
    Alu = mybir.AluOpType
    Act = mybir.ActivationFunctionType

    nc = bacc.Bacc("TRN2", target_bir_lowering=False, num_devices=NC)

    # ---- inputs ----
    xT = nc.dram_tensor("xT", [FIN, NPADA], f32, kind="ExternalInput")
    xTown = nc.dram_tensor("xTown", [FIN, NLOC], f32, kind="ExternalInput")
    w1e = nc.dram_tensor("w1e", [FIN, 264], f32, kind="ExternalInput")   # [W1|Wss1|Wsd1]
    wsd_own = nc.dram_tensor("wsd_own", [FIN, 4], f32, kind="ExternalInput")  # Wsd1
    wmue = nc.dram_tensor("wmue", [HID, 130], f32, kind="ExternalInput")  # [Wmu|vmu|umu]
    wlve = nc.dram_tensor("wlve", [HID, 130], f32, kind="ExternalInput")
    b1b = nc.dram_tensor("b1b", [128, 256], f32, kind="ExternalInput")
    bmub = nc.dram_tensor("bmub", [128, 128], f32, kind="ExternalInput")
    blvb = nc.dram_tensor("blvb", [128, 128], f32, kind="ExternalInput")
    iota = nc.dram_tensor("iota", [128, 128], f32, kind="ExternalInput")
    ident = nc.dram_tensor("ident", [128, 128], f32, kind="ExternalInput")
    srcg = nc.dram_tensor("srcg", [128, EPAD // 16], i16, kind="ExternalInput")
    src2 = nc.dram_tensor("src2", [128, EPAD // 16], i16, kind="ExternalInput")
    dstl = nc.dram_tensor("dstl", [128, EPAD // 16], i16, kind="ExternalInput")
    dstoffT = nc.dram_tensor("dstoffT", [128, TILES], f32, kind="ExternalInput")
    wT = nc.dram_tensor("wT", [128, TILES], f32, kind="ExternalInput")
    winvT = nc.dram_tensor("winvT", [128, TILES], f32, kind="ExternalInput")

    mu_out = nc.dram_tensor("mu_out", [NLOC, LAT], f32, kind="ExternalOutput")
    lv_out = nc.dram_tensor("lv_out", [NLOC, LAT], f32, kind="ExternalOutput")
    dbg_xpt = nc.dram_tensor("dbg_xpt", [128, 260], f32, kind="ExternalOutput")
    dbg_extd = nc.dram_tensor("dbg_extd", [128, 6], f32, kind="ExternalOutput")
    dbg_agin = nc.dram_tensor("dbg_agin", [128, 258], f32, kind="ExternalOutput")
    dbg_agout = nc.dram_tensor("dbg_agout", [128, 258], f32, kind="ExternalOutput")
    dbg_xrow = nc.dram_tensor("dbg_xrow", [128, 8, XW], f32, kind="ExternalOutput")
    dbg_erow = nc.dram_tensor("dbg_erow", [128, 8, EXW_], f32, kind="ExternalOutput")
    dbg_exw = nc.dram_tensor("dbg_exw", [128, 8, 4], f32, kind="ExternalOutput")
    dbg_ps = nc.dram_tensor("dbg_ps", [128, 260], f32, kind="ExternalOutput")
    dbg_hb = nc.dram_tensor("dbg_hb", [128, 256], f32, kind="ExternalOutput")
    dbg_oh = nc.dram_tensor("dbg_oh", [128, 128], f32, kind="ExternalOutput")

    with tile.TileContext(nc) as tc:
        with (
            tc.tile_pool(name="cst", bufs=1) as cst,
            tc.tile_pool(name="lw", bufs=3) as lw,
            tc.tile_pool(name="xa", bufs=3) as xa,
            tc.tile_pool(name="gx", bufs=3) as gx,
            tc.tile_pool(name="ge", bufs=3) as ge,
            tc.tile_pool(name="oh", bufs=6) as ohp,
            tc.tile_pool(name="sm", bufs=4) as sm,
            tc.tile_pool(name="fin", bufs=3) as fin,
            tc.tile_pool(name="ps2", bufs=2, space="PSUM") as ps2,
            tc.tile_pool(name="ps1", bufs=1, space="PSUM") as ps1,
            tc.tile_pool(name="dr", bufs=1, space="DRAM") as dr,
        ):
            # internal DRAM tables (pool tiles so Tile tracks RAW deps)
            XPT = dr.tile([NPADA, XW], f32, tag="XPT")
            EXTD = dr.tile([NLOC, EXW_], f32, tag="EXTD")

            # resident constants
            w1e_t = []
            for kk in range(4):
                t = cst.tile([128, 264], f32, tag=f"w1e{kk}")
                nc.sync.dma_start(t[:], w1e[kk * 128:(kk + 1) * 128, :])
                w1e_t.append(t)
            wsd_t = []
            for kk in range(4):
                t = cst.tile([128, 4], f32, tag=f"wsd{kk}")
                nc.sync.dma_start(t[:], wsd_own[kk * 128:(kk + 1) * 128, :])
                wsd_t.append(t)
            wmue_t = []
            wlve_t = []
            for kk in range(2):
                t = cst.tile([128, 130], f32, tag=f"wmue{kk}")
                nc.sync.dma_start(t[:], wmue[kk * 128:(kk + 1) * 128, :])
                wmue_t.append(t)
                t2 = cst.tile([128, 130], f32, tag=f"wlve{kk}")
                nc.sync.dma_start(t2[:], wlve[kk * 128:(kk + 1) * 128, :])
                wlve_t.append(t2)
            b1b_t = cst.tile([128, 256], f32, tag="b1b")
            nc.sync.dma_start(b1b_t[:], b1b[:])
            bmub_t = cst.tile([128, 128], f32, tag="bmub")
            nc.sync.dma_start(bmub_t[:], bmub[:])
            blvb_t = cst.tile([128, 128], f32, tag="blvb")
            nc.sync.dma_start(blvb_t[:], blvb[:])
            iota_t = cst.tile([128, 128], f32, tag="iota")
            nc.sync.dma_start(iota_t[:], iota[:])
            ident_t = cst.tile([128, 128], f32, tag="ident")
            nc.sync.dma_start(ident_t[:], ident[:])
            srcg_t = cst.tile([128, EPAD // 16], i16, tag="srcg")
            nc.sync.dma_start(srcg_t[:], srcg[:])
            src2_t = cst.tile([128, EPAD // 16], i16, tag="src2")
            nc.sync.dma_start(src2_t[:], src2[:])
            dstl_t = cst.tile([128, EPAD // 16], i16, tag="dstl")
            nc.sync.dma_start(dstl_t[:], dstl[:])
            dstoffT_t = cst.tile([128, TILES], f32, tag="dstoffT")
            nc.sync.dma_start(dstoffT_t[:], dstoffT[:])
            wT_t = cst.tile([128, TILES], f32, tag="wT")
            nc.sync.dma_start(wT_t[:], wT[:])
            winvT_t = cst.tile([128, TILES], f32, tag="winvT")
            nc.sync.dma_start(winvT_t[:], winvT[:])

            # ---- phase A: XPT[n] = [xp(256) | ss1(4)] for all nodes ----
            for t in range(NPADA // 128):
                ps = ps2.tile([128, 264], f32, tag="blk")
                for kk in range(4):
                    lx = lw.tile([128, 128], f32, tag="lx")
                    nc.sync.dma_start(lx[:], xT[kk * 128:(kk + 1) * 128,
                                                 t * 128:(t + 1) * 128])
                    nc.tensor.matmul(ps[:], lx[:], w1e_t[kk][:],
                                     start=(kk == 0), stop=(kk == 3))
                xps = xa.tile([128, 260], f32, tag="xps")
                nc.scalar.copy(xps[:], ps[:, 0:260])
                nc.sync.dma_start(XPT[t * 128:(t + 1) * 128, 0:260], xps[:])

            # ---- phase A-own: sd1 for own nodes -> EXTD[:,0:4] ----
            for b in range(BLOCKS):
                ps = ps2.tile([128, 264], f32, tag="blk")
                for kk in range(4):
                    lx = lw.tile([128, 128], f32, tag="lx")
                    nc.sync.dma_start(lx[:], xTown[kk * 128:(kk + 1) * 128,
                                                   b * 128:(b + 1) * 128])
                    nc.tensor.matmul(ps[:, 0:4], lx[:], wsd_t[kk][:],
                                     start=(kk == 0), stop=(kk == 3))
                sds = xa.tile([128, 4], f32, tag="sds")
                nc.scalar.copy(sds[:], ps[:, 0:4])
                nc.sync.dma_start(EXTD[b * 128:(b + 1) * 128, 0:4], sds[:])

            # AllGather buffers for layer-2/3 message rows
            agin = dr.tile([NLOC, X2W], f32)
            agout = dr.tile([NC * NLOC, X2W], f32, addr_space="Shared")

            # ---- L1 edge phase + finalize ----
            blk_ps = {}
            for ci in range(NCHUNK):
                xrow = gx.tile([128, TPC, XW], f32, tag="xrow")
                nc.gpsimd.dma_gather(
                    xrow[:], XPT[:], srcg_t[:, ci * 64:(ci + 1) * 64],
                    CHUNK, CHUNK, XW)
                extd = ge.tile([128, TPC, EXW_], f32, tag="extd")
                nc.gpsimd.dma_gather(
                    extd[:], EXTD[:], dstl_t[:, ci * 64:(ci + 1) * 64],
                    CHUNK, CHUNK, EXW_)
                if ci == 0:
                    nc.sync.dma_start(dbg_xrow[:], xrow[:])
                    nc.sync.dma_start(dbg_erow[:], extd[:])
                # alpha for the whole chunk: z=ss+sd, leaky, exp, *w
                z = sm.tile([128, TPC, 4], f32, tag="z")
                nc.vector.tensor_tensor(z[:], xrow[:, :, 256:260],
                                        extd[:, :, 0:4], op=Alu.add)
                nc.vector.scalar_tensor_tensor(
                    z[:], in0=z[:], scalar=NEG, in1=z[:],
                    op0=Alu.mult, op1=Alu.max)
                ex = sm.tile([128, TPC, 4], f32, tag="ex")
                nc.scalar.activation(ex[:], z[:], Act.Exp)
                exw = sm.tile([128, TPC, 4], f32, tag="exw")
                wb = wT_t[:, ci * TPC:(ci + 1) * TPC]
                nc.vector.tensor_tensor(
                    exw[:], ex[:], wb.rearrange("p (t o) -> p t o", o=1).to_broadcast([128, TPC, 4]), op=Alu.mult)

                if ci == 0:
                    nc.sync.dma_start(dbg_exw[:], exw[:])
                for tt in range(TPC):
                    t = ci * TPC + tt
                    b = t // TPB
                    k = t % TPB
                    if k == 0:
                        blk_ps[b] = ps2.tile([128, 264], f32, tag="blk", name="blkps")
                    ps = blk_ps[b]
                    for h in range(H):
                        ohx = ohp.tile([128, 128], f32, tag="ohx")
                        nc.vector.tensor_scalar(
                            ohx[:], iota_t[:], dstoffT_t[:, t:t + 1],
                            exw[:, tt, h:h + 1], Alu.is_equal, Alu.mult)
                        if t == 0 and h == 0:
                            nc.sync.dma_start(dbg_oh[:], ohx[:])
                        nc.tensor.matmul(
                            ps[:, h * 64:(h + 1) * 64], ohx[:],
                            xrow[:, tt, h * 64:(h + 1) * 64],
                            start=(k == 0 and h == 0), stop=False,
                            skip_group_check=True)
                        nc.tensor.matmul(
                            ps[:, 256 + h:257 + h], ohx[:],
                            winvT_t[:, t:t + 1],
                            start=False,
                            stop=(k == TPB - 1 and h == H - 1),
                            skip_group_check=True)
                    if k == TPB - 1:
                        if b == 0:
                            pscp = fin.tile([128, 260], f32, tag="pscp")
                            nc.scalar.copy(pscp[:], ps[:, 0:260])
                            nc.sync.dma_start(dbg_ps[:], pscp[:])
                        # finalize block b -> h, then next-layer rows
                        den = sm.tile([128, 4], f32, tag="den")
                        nc.vector.tensor_scalar_add(den[:], ps[:, 256:260], EPS)
                        rec = sm.tile([128, 4], f32, tag="rec")
                        nc.vector.reciprocal(rec[:], den[:])
                        hb = fin.tile([128, 256], f32, tag="hb")
                        for h in range(H):
                            nc.vector.scalar_tensor_tensor(
                                hb[:, h * 64:(h + 1) * 64],
                                in0=ps[:, h * 64:(h + 1) * 64],
                                scalar=rec[:, h:h + 1],
                                in1=b1b_t[:, h * 64:(h + 1) * 64],
                                op0=Alu.mult, op1=Alu.add)
                        # ELU: h = max(z,0) + exp(min(z,0)) - 1
                        zm = fin.tile([128, 256], f32, tag="zm")
                        nc.vector.tensor_scalar_min(zm[:], hb[:], 0.0)
                        ez = fin.tile([128, 256], f32, tag="ez")
                        nc.scalar.activation(ez[:], zm[:], Act.Exp)
                        nc.vector.scalar_tensor_tensor(
                            hb[:], in0=hb[:], scalar=0.0, in1=ez[:],
                            op0=Alu.max, op1=Alu.add)
                        nc.vector.tensor_scalar_add(hb[:], hb[:], -1.0)
                        if b == 0:
                            nc.sync.dma_start(dbg_hb[:], hb[:])
                        # transpose h (2 x 128x128) then project to mu/lv rows
                        hTs = []
                        for half in range(2):
                            pst = ps1.tile([128, 128], f32, tag="pst")
                            nc.tensor.transpose(
                                pst[:], hb[:, half * 128:(half + 1) * 128],
                                ident_t[:])
                            hT = fin.tile([128, 128], f32, tag=f"hT{half}")
                            nc.vector.tensor_copy(hT[:], pst[:])
                            hTs.append(hT)
                        psmu = ps1.tile([128, 130], f32, tag="psmu")
                        pslv = ps1.tile([128, 130], f32, tag="pslv")
                        for kk in range(2):
                            nc.tensor.matmul(psmu[:], hTs[kk][:], wmue_t[kk][:],
                                             start=(kk == 0), stop=(kk == 1))
                            nc.tensor.matmul(pslv[:], hTs[kk][:], wlve_t[kk][:],
                                             start=(kk == 0), stop=(kk == 1))
                        xr2 = fin.tile([128, 258], f32, tag="xr2")
                        nc.scalar.copy(xr2[:, 0:128], psmu[:, 0:128])
                        nc.scalar.copy(xr2[:, 128:256], pslv[:, 0:128])
                        nc.scalar.copy(xr2[:, 256:257], psmu[:, 128:129])
                        nc.scalar.copy(xr2[:, 257:258], pslv[:, 128:129])
                        nc.sync.dma_start(
                            agin[b * 128:(b + 1) * 128, 0:258], xr2[:])
                        sd2 = sm.tile([128, 2], f32, tag="sd2")
                        nc.scalar.copy(sd2[:, 0:1], psmu[:, 129:130])
                        nc.scalar.copy(sd2[:, 1:2], pslv[:, 129:130])
                        nc.sync.dma_start(
                            EXTD[b * 128:(b + 1) * 128, 4:6], sd2[:])
                        del blk_ps[b]

            # ---- exchange layer-2/3 message rows ----
            nc.gpsimd.collective_compute(
                "AllGather", mybir.AluOpType.bypass,
                replica_groups=[list(range(NC))],
                ins=[agin.opt()], outs=[agout.opt()])

            # ---- L2/3 edge phase (mu and lv share gathers) ----
            blk2 = {}
            for ci in range(NCHUNK):
                xrow = gx.tile([128, TPC, X2W], f32, tag="xrow2")
                nc.gpsimd.dma_gather(
                    xrow[:], agout[:], src2_t[:, ci * 64:(ci + 1) * 64],
                    CHUNK, CHUNK, X2W)
                extd = ge.tile([128, TPC, EXW_], f32, tag="extd2")
                nc.gpsimd.dma_gather(
                    extd[:], EXTD[:], dstl_t[:, ci * 64:(ci + 1) * 64],
                    CHUNK, CHUNK, EXW_)
                z = sm.tile([128, TPC, 2], f32, tag="z2")
                nc.vector.tensor_tensor(z[:], xrow[:, :, 256:258],
                                        extd[:, :, 4:6], op=Alu.add)
                nc.vector.scalar_tensor_tensor(
                    z[:], in0=z[:], scalar=NEG, in1=z[:],
                    op0=Alu.mult, op1=Alu.max)
                ex = sm.tile([128, TPC, 2], f32, tag="ex2")
                nc.scalar.activation(ex[:], z[:], Act.Exp)
                exw = sm.tile([128, TPC, 2], f32, tag="exw2")
                wb = wT_t[:, ci * TPC:(ci + 1) * TPC]
                nc.vector.tensor_tensor(
                    exw[:], ex[:], wb.rearrange("p (t o) -> p t o", o=1).to_broadcast([128, TPC, 2]), op=Alu.mult)

                for tt in range(TPC):
                    t = ci * TPC + tt
                    b = t // TPB
                    k = t % TPB
                    if k == 0:
                        blk2[b] = (ps1.tile([128, 129], f32, tag="bmu", name="bmups"),
                                   ps1.tile([128, 129], f32, tag="blv", name="blvps"))
                    pmu, plv = blk2[b]
                    for li, ps2 in enumerate((pmu, plv)):
                        ohx = ohp.tile([128, 128], f32, tag="ohx")
                        nc.vector.tensor_scalar(
                            ohx[:], iota_t[:], dstoffT_t[:, t:t + 1],
                            exw[:, tt, li:li + 1], Alu.is_equal, Alu.mult)
                        nc.tensor.matmul(
                            ps2[:, 0:128], ohx[:],
                            xrow[:, tt, li * 128:(li + 1) * 128],
                            start=(k == 0), stop=False,
                            skip_group_check=True)
                        nc.tensor.matmul(
                            ps2[:, 128:129], ohx[:], winvT_t[:, t:t + 1],
                            start=False, stop=(k == TPB - 1),
                            skip_group_check=True)
                    if k == TPB - 1:
                        for li, (ps2, bias_t, outdr) in enumerate(
                                ((pmu, bmub_t, mu_out), (plv, blvb_t, lv_out))):
                            den = sm.tile([128, 1], f32, tag="den2")
                            nc.vector.tensor_scalar_add(
                                den[:], ps2[:, 128:129], EPS)
                            rec = sm.tile([128, 1], f32, tag="rec2")
                            nc.vector.reciprocal(rec[:], den[:])
                            ob = fin.tile([128, 128], f32, tag="ob")
                            nc.vector.scalar_tensor_tensor(
                                ob[:], in0=ps2[:, 0:128], scalar=rec[:, 0:1],
                                in1=bias_t[:], op0=Alu.mult, op1=Alu.add)
                            nc.sync.dma_start(
                                outdr[b * 128:(b + 1) * 128, :], ob[:])
                        del blk2[b]

            nc.sync.dma_start(dbg_xpt[:], XPT[0:128, 0:260])
            nc.sync.dma_start(dbg_extd[:], EXTD[0:128, 0:6])
            nc.sync.dma_start(dbg_agin[:], agin[0:128, 0:258])
            nc.sync.dma_start(dbg_agout[:], agout[0:128, 0:258])

    nc.compile()
    return nc


def _prep_inputs(x, edge_index, edge_weight, W1, att1, b1, Wmu, attmu, bmu,
                 Wlv, attlv, blv):
    src = np.asarray(edge_index[0], np.int64)
    dst = np.asarray(edge_index[1], np.int64)
    w = np.asarray(edge_weight, np.float32)
    x = np.asarray(x, np.float32)

    # fused weights
    att1 = np.asarray(att1, np.float32)          # [H, 2*C1]
    a_dst = att1[:, :C1]                          # xi (dst) half
    a_src = att1[:, C1:]                          # xj (src) half
    W1 = np.asarray(W1, np.float32)
    Wss1 = np.zeros((FIN, H), np.float32)
    Wsd1 = np.zeros((FIN, H), np.float32)
    for h in range(H):
        Wss1[:, h] = W1[:, h * C1:(h + 1) * C1] @ a_src[h]
        Wsd1[:, h] = W1[:, h * C1:(h + 1) * C1] @ a_dst[h]
    w1e = np.concatenate([W1, Wss1, Wsd1], axis=1)          # [512, 264]

    attmu = np.asarray(attmu, np.float32).reshape(-1)        # [2*LAT]
    attlv = np.asarray(attlv, np.float32).reshape(-1)
    Wmu = np.asarray(Wmu, np.float32)
    Wlv = np.asarray(Wlv, np.float32)
    wmue = np.concatenate(
        [Wmu, (Wmu @ attmu[LAT:])[:, None], (Wmu @ attmu[:LAT])[:, None]], axis=1)
    wlve = np.concatenate(
        [Wlv, (Wlv @ attlv[LAT:])[:, None], (Wlv @ attlv[:LAT])[:, None]], axis=1)

    xT = np.zeros((FIN, NPADA), np.float32)
    xT[:, :N] = x.T
    b1b = np.tile(np.asarray(b1, np.float32)[None, :], (128, 1))
    bmub = np.tile(np.asarray(bmu, np.float32)[None, :], (128, 1))
    blvb = np.tile(np.asarray(blv, np.float32)[None, :], (128, 1))
    iota = np.tile(np.arange(128, dtype=np.float32)[None, :], (128, 1))
    ident = np.eye(128, dtype=np.float32)

    # sort edges by dst, bucket per core, pad per 128-node block to TPB tiles
    order = np.argsort(dst, kind="stable")
    ssrc, sdst, sw = src[order], dst[order], w[order]
    core_of = sdst // NOWN
    in_maps = []
    for c in range(NC):
        m = core_of == c
        cs, cd, cw = ssrc[m], sdst[m] - c * NOWN, sw[m]
        blk = cd // 128
        e_src = np.zeros(EPAD, np.int64)
        e_dstloc = np.zeros(EPAD, np.int64)
        e_dstoff = np.full(EPAD, -1.0, np.float32)
        e_w = np.zeros(EPAD, np.float32)
        e_winv = np.ones(EPAD, np.float32)
        for b in range(BLOCKS):
            bm = blk == b
            nbe = int(bm.sum())
            if nbe > TPB * 128:
                raise RuntimeError(f"block overflow core {c} block {b}: {nbe}")
            o = b * TPB * 128
            e_src[o:o + nbe] = cs[bm]
            e_dstloc[o:o + nbe] = cd[bm]
            e_dstoff[o:o + nbe] = (cd[bm] - b * 128).astype(np.float32)
            wcl = np.maximum(cw[bm], 1e-30)
            e_w[o:o + nbe] = wcl
            e_winv[o:o + nbe] = 1.0 / wcl
        own = e_src // NOWN
        e_src2 = own * NLOC + (e_src - own * NOWN)
        xTown = np.zeros((FIN, NLOC), np.float32)
        xTown[:, :NOWN] = x.T[:, c * NOWN:(c + 1) * NOWN]
        in_maps.append({
            "xT": xT, "xTown": xTown, "w1e": w1e, "wsd_own": Wsd1,
            "wmue": wmue, "wlve": wlve, "b1b": b1b, "bmub": bmub,
            "blvb": blvb, "iota": iota, "ident": ident,
            "srcg": _wrap_idxs(e_src), "src2": _wrap_idxs(e_src2),
            "dstl": _wrap_idxs(e_dstloc),
            "dstoffT": _colmajor(e_dstoff), "wT": _colmajor(e_w),
            "winvT": _colmajor(e_winv),
        })
    return in_maps


def kernel(x, edge_index, edge_weight, W1, att1, b1, Wmu, attmu, bmu,
           Wlv, attlv, blv):
    from concourse.bass_utils import run_bass_kernel_spmd

    if "nc" not in _cache:
        _cache["nc"] = _build_module()
    nc = _cache["nc"]
    in_maps = _prep_inputs(x, edge_index, edge_weight, W1, att1, b1,
                           Wmu, attmu, bmu, Wlv, attlv, blv)
    r = run_bass_kernel_spmd(nc, in_maps, list(range(NC)))
    mu = np.zeros((N, LAT), np.float32)
    lv = np.zeros((N, LAT), np.float32)
    for c in range(NC):
        mu[c * NOWN:(c + 1) * NOWN] = r.results[c]["mu_out"][:NOWN]
        lv[c * NOWN:(c + 1) * NOWN] = r.results[c]["lv_out"][:NOWN]
    return (mu, lv)


# revision 15
# speedup vs baseline: 1.6633x; 1.6633x over previous
"""GAT encoder (3 GAT layers: 256-hid 4-head concat + mu/logvar 128) on 8 trn2 cores.

Strategy (dst-range node sharding, per sharding_hint):
 - Host sorts edges by dst, buckets per core (2500 dst nodes each), pads each
   128-node block to TPB tiles of 128 edges.
 - Phase A (per core, redundant): xp = x @ W1 (bf16 matmul) for all nodes plus
   an f32 correction matmul for the fused per-node logit columns
   (Wss1[f,h] = sum_c W1[f,h*64+c]*att_src[h,c]); rows -> DRAM table XPT
   (bf16 payload + f32 logit bytes).
 - L1 edge phase: dma_gather xp rows by src, dma_gather dst logits; softmax
   coefs scaled into the gathered rows in place; one plain one-hot (bf16) +
   one wide PSUM-accumulated matmul per 128-edge tile aggregates messages and
   denominators (ex column).
 - L1 finalize per block: normalize, +bias, ELU -> h; PE-transpose h and
   matmul with [Wmu|vmu|umu]/[Wlv|vlv|ulv] to get next-layer message rows
   xpmu/xplv + logits; rows go into an AllGather across the 8 cores.
 - L2/3 edge phase: same staircase trick, H=1, mu and lv share one gather and
   one matmul per tile.
Outputs (mu, logvar) assembled host-side from per-core slices.
"""

import numpy as np

# ---- problem constants (hardcoded per contract) ----
N = 20000
E = 320000
FIN = 512
HID = 256
LAT = 128
H = 4
C1 = 64
NEG = 0.2
EPS = 1e-16

NC = 8
NOWN = 2500          # dst nodes per core
BLOCKS = 20          # 128-node blocks per core (2560 padded local nodes)
NLOC = BLOCKS * 128  # 2560
TPB = 17             # tiles (128 edges) per block (max real block = 2174 edges)
TILES = BLOCKS * TPB       # 340 real tiles
TPC = 8                    # tiles per gather chunk
CHUNK = TPC * 128          # 1024 idxs per dma_gather (hw limit ~1024)
TILES_PAD = 344            # pad to full chunks
NCHUNK = TILES_PAD // TPC  # 43
EPAD = TILES_PAD * 128     # 44032 edge slots per core
NPADA = 160 * 128          # 20480 padded global rows (divisible by 512)
XW = 384                   # XPT row bf16 width (768B): xp 0:256, ss1 f32 @bytes 512:528
X2W = 384                  # XPT2 row: xpmu 0:128, xplv 128:256, ssmu/sslv f32 @bytes 512:520
EXW_ = 64                  # EXTD row f32 width (sd1 0:4, sdmu 4, sdlv 5)

_cache = {}


def _wrap_idxs(idx):
    n = idx.shape[0]
    t = np.zeros((128, n // 16), np.int16)
    w = idx.reshape(n // 16, 16).T.astype(np.int16)
    for g in range(8):
        t[g * 16:(g + 1) * 16, :] = w
    return t


def _colmajor(a):
    # per-edge array [EPAD] -> [128, TILES_PAD] tile-column layout
    return np.ascontiguousarray(a.reshape(TILES_PAD, 128).T)


def _build_module(upto="full"):
    import concourse.bacc as bacc
    import concourse.mybir as mybir
    import concourse.tile as tile

    f32 = mybir.dt.float32
    bf16 = mybir.dt.bfloat16
    i16 = mybir.dt.int16
    Alu = mybir.AluOpType
    Act = mybir.ActivationFunctionType

    nc = bacc.Bacc("TRN2", target_bir_lowering=False, num_devices=NC)

    # ---- inputs ----
    xT = nc.dram_tensor("xT", [FIN, NPADA], f32, kind="ExternalInput")
    xTown = nc.dram_tensor("xTown", [FIN, NLOC], f32, kind="ExternalInput")
    w1b = nc.dram_tensor("w1b", [FIN, 256], bf16, kind="ExternalInput")
    wse = nc.dram_tensor("wse", [FIN, 8], f32, kind="ExternalInput")   # [Wss1|Wsd1]
    wsd_own = nc.dram_tensor("wsd_own", [FIN, 4], f32, kind="ExternalInput")
    wmue = nc.dram_tensor("wmue", [HID, 130], bf16, kind="ExternalInput")  # [Wmu|vmu|umu]
    wlve = nc.dram_tensor("wlve", [HID, 130], bf16, kind="ExternalInput")
    b1b = nc.dram_tensor("b1b", [128, 256], f32, kind="ExternalInput")
    bmub = nc.dram_tensor("bmub", [128, 128], f32, kind="ExternalInput")
    blvb = nc.dram_tensor("blvb", [128, 128], f32, kind="ExternalInput")
    iota = nc.dram_tensor("iota", [128, 128], bf16, kind="ExternalInput")
    ident = nc.dram_tensor("ident", [128, 128], f32, kind="ExternalInput")
    srcg = nc.dram_tensor("srcg", [128, EPAD // 16], i16, kind="ExternalInput")
    src2 = nc.dram_tensor("src2", [128, EPAD // 16], i16, kind="ExternalInput")
    dstl = nc.dram_tensor("dstl", [128, EPAD // 16], i16, kind="ExternalInput")
    dstoffT = nc.dram_tensor("dstoffT", [128, TILES_PAD], f32, kind="ExternalInput")
    wT = nc.dram_tensor("wT", [128, TILES_PAD], f32, kind="ExternalInput")

    mu_out = nc.dram_tensor("mu_out", [NLOC, LAT], f32, kind="ExternalOutput")
    lv_out = nc.dram_tensor("lv_out", [NLOC, LAT], f32, kind="ExternalOutput")
    dbg_xpt = nc.dram_tensor("dbg_xpt", [128, 264], mybir.dt.bfloat16, kind="ExternalOutput")
    dbg_extd = nc.dram_tensor("dbg_extd", [128, 6], f32, kind="ExternalOutput")
    dbg_z = nc.dram_tensor("dbg_z", [128, 8, 4], f32, kind="ExternalOutput")
    dbg_ps = nc.dram_tensor("dbg_ps", [128, 260], f32, kind="ExternalOutput")
    dbg_hb = nc.dram_tensor("dbg_hb", [128, 256], f32, kind="ExternalOutput")
    dbg_agin = nc.dram_tensor("dbg_agin", [128, 260], mybir.dt.bfloat16, kind="ExternalOutput")

    with tile.TileContext(nc) as tc:
        with (
            tc.tile_pool(name="cst", bufs=1) as cst,
            tc.tile_pool(name="lw", bufs=3) as lw,
            tc.tile_pool(name="xa", bufs=3) as xa,
            tc.tile_pool(name="gx", bufs=3) as gx,
            tc.tile_pool(name="ge", bufs=3) as ge,
            tc.tile_pool(name="oh", bufs=6) as ohp,
            tc.tile_pool(name="sm", bufs=4) as sm,
            tc.tile_pool(name="fin", bufs=3) as fin,
            tc.tile_pool(name="ps2", bufs=2, space="PSUM") as ps2,
            tc.tile_pool(name="ps1", bufs=1, space="PSUM") as ps1,
            tc.tile_pool(name="dr", bufs=1, space="DRAM") as dr,
        ):
            # internal DRAM tables (pool tiles so Tile tracks RAW deps)
            XPT = dr.tile([NPADA, XW], bf16, tag="XPT")
            EXTD = dr.tile([NLOC, EXW_], f32, tag="EXTD")

            # resident constants
            w1b_t = []
            for kk in range(4):
                t = cst.tile([128, 256], bf16, tag=f"w1b{kk}")
                nc.sync.dma_start(t[:], w1b[kk * 128:(kk + 1) * 128, :])
                w1b_t.append(t)
            wse_t = []
            for kk in range(4):
                t = cst.tile([128, 8], f32, tag=f"wse{kk}")
                nc.sync.dma_start(t[:], wse[kk * 128:(kk + 1) * 128, :])
                wse_t.append(t)
            wsd_t = []
            for kk in range(4):
                t = cst.tile([128, 4], f32, tag=f"wsd{kk}")
                nc.sync.dma_start(t[:], wsd_own[kk * 128:(kk + 1) * 128, :])
                wsd_t.append(t)
            wmue_t = []
            wlve_t = []
            for kk in range(2):
                t = cst.tile([128, 130], bf16, tag=f"wmue{kk}")
                nc.sync.dma_start(t[:], wmue[kk * 128:(kk + 1) * 128, :])
                wmue_t.append(t)
                t2 = cst.tile([128, 130], bf16, tag=f"wlve{kk}")
                nc.sync.dma_start(t2[:], wlve[kk * 128:(kk + 1) * 128, :])
                wlve_t.append(t2)
            b1b_t = cst.tile([128, 256], f32, tag="b1b")
            nc.sync.dma_start(b1b_t[:], b1b[:])
            bmub_t = cst.tile([128, 128], f32, tag="bmub")
            nc.sync.dma_start(bmub_t[:], bmub[:])
            blvb_t = cst.tile([128, 128], f32, tag="blvb")
            nc.sync.dma_start(blvb_t[:], blvb[:])
            iota_t = cst.tile([128, 128], bf16, tag="iota")
            nc.sync.dma_start(iota_t[:], iota[:])
            ident_t = cst.tile([128, 128], f32, tag="ident")
            nc.sync.dma_start(ident_t[:], ident[:])
            srcg_t = cst.tile([128, EPAD // 16], i16, tag="srcg")
            nc.sync.dma_start(srcg_t[:], srcg[:])
            src2_t = cst.tile([128, EPAD // 16], i16, tag="src2")
            nc.sync.dma_start(src2_t[:], src2[:])
            dstl_t = cst.tile([128, EPAD // 16], i16, tag="dstl")
            nc.sync.dma_start(dstl_t[:], dstl[:])
            dstoffT_t = cst.tile([128, TILES_PAD], f32, tag="dstoffT")
            nc.sync.dma_start(dstoffT_t[:], dstoffT[:])
            wT_t = cst.tile([128, TILES_PAD], f32, tag="wT")
            nc.sync.dma_start(wT_t[:], wT[:])

            # ---- phase A: XPT[n] = [xp bf16 (256) | ss1 f32 (4)] ----
            for g in range(NPADA // 512):
                lxs, lxbs = [], []
                for kk in range(4):
                    lxk = lw.tile([128, 512], f32, tag=f"lx{kk}")
                    nc.scalar.dma_start(
                        lxk[:], xT[kk * 128:(kk + 1) * 128,
                                    512 * g:512 * (g + 1)])
                    lxs.append(lxk)
                    lxb = lw.tile([128, 512], bf16, tag=f"lxb{kk}")
                    nc.scalar.copy(lxb[:], lxk[:])
                    lxbs.append(lxb)
                xps = xa.tile([128, 4, 264], bf16, tag="xps")
                for ti in range(4):
                    ps = ps2.tile([128, 256], f32, tag="blk", name="psA")
                    pss = ps1.tile([128, 8], f32, tag="pslog", name="psAl")
                    for kk in range(4):
                        sl = slice(ti * 128, (ti + 1) * 128)
                        nc.tensor.matmul(ps[:], lxbs[kk][:, sl], w1b_t[kk][:],
                                         start=(kk == 0), stop=(kk == 3))
                        nc.tensor.matmul(pss[:], lxs[kk][:, sl], wse_t[kk][:],
                                         start=(kk == 0), stop=(kk == 3))
                    nc.scalar.copy(xps[:, ti, 0:256], ps[:])
                    nc.vector.tensor_copy(
                        xps[:, ti, 256:264].bitcast(f32), pss[:, 0:4])
                nc.sync.dma_start(
                    XPT[:].rearrange("(g4 p) c -> p g4 c", p=128)
                    [:, 4 * g:4 * g + 4, 0:264], xps[:])

            # ---- phase A-own: sd1 for own nodes -> EXTD[:,0:4] ----
            for b in range(BLOCKS):
                lx = lw.tile([128, 4, 128], f32, tag="lxo")
                nc.scalar.dma_start(
                    lx[:], xTown[:].rearrange("(kk p) (b j) -> p kk b j",
                                              p=128, j=128)[:, :, b, :])
                ps = ps1.tile([128, 8], f32, tag="pslog", name="psOwn")
                for kk in range(4):
                    nc.tensor.matmul(ps[:, 0:4], lx[:, kk, :], wsd_t[kk][:],
                                     start=(kk == 0), stop=(kk == 3))
                sds = xa.tile([128, 4], f32, tag="sds")
                nc.scalar.copy(sds[:], ps[:, 0:4])
                nc.sync.dma_start(EXTD[b * 128:(b + 1) * 128, 0:4], sds[:])

            # AllGather buffers for layer-2/3 message rows
            agin = dr.tile([NLOC, X2W], bf16, tag="agin")
            agout = dr.tile([NC * NLOC, X2W], bf16, tag="agout",
                            addr_space="Shared")

            # ---- L1 edge phase + finalize ----
            blk_ps = {}
            for ci in range(NCHUNK if upto != "A" else 0):
                xrow = gx.tile([128, TPC, XW], bf16, tag="xrow")
                nc.gpsimd.dma_gather(
                    xrow[:], XPT[:], srcg_t[:, ci * 64:(ci + 1) * 64],
                    CHUNK, CHUNK, XW)
                extd = ge.tile([128, TPC, EXW_], f32, tag="extd")
                nc.gpsimd.dma_gather(
                    extd[:], EXTD[:], dstl_t[:, ci * 64:(ci + 1) * 64],
                    CHUNK, CHUNK, EXW_)
                # alpha for the whole chunk: z=ss+sd, leaky, exp, *w
                z = sm.tile([128, TPC, 4], f32, tag="z")
                nc.vector.tensor_tensor(z[:], xrow[:, :, 256:264].bitcast(f32),
                                        extd[:, :, 0:4], op=Alu.add)
                nc.vector.scalar_tensor_tensor(
                    z[:], in0=z[:], scalar=NEG, in1=z[:],
                    op0=Alu.mult, op1=Alu.max)
                if ci == 0:
                    nc.sync.dma_start(dbg_z[:], z[:])
                ex = sm.tile([128, TPC, 4], f32, tag="ex")
                nc.scalar.activation(ex[:], z[:], Act.Exp)
                exw = sm.tile([128, TPC, 4], f32, tag="exw")
                wb = wT_t[:, ci * TPC:(ci + 1) * TPC]
                nc.vector.tensor_tensor(
                    exw[:], ex[:],
                    wb.rearrange("p (t o) -> p t o", o=1).to_broadcast(
                        [128, TPC, 4]), op=Alu.mult)
                exwb = sm.tile([128, TPC, 4], bf16, tag="exwb")
                nc.vector.tensor_copy(exwb[:], exw[:])
                # scale message cols in place: heads 0,1 on DVE (one bcast op),
                # heads 2,3 on ACT (per-tile per-head scaled copies)
                xr01 = xrow[:, :, 0:128].rearrange("p t (h c) -> p t h c", c=64)
                nc.vector.tensor_tensor(
                    xr01, xr01,
                    exwb[:, :, 0:2].rearrange("p t (h o) -> p t h o", o=1)
                    .to_broadcast([128, TPC, 2, 64]), op=Alu.mult)
                for tt in range(TPC):
                    for h in (2, 3):
                        nc.scalar.mul(xrow[:, tt, h * 64:(h + 1) * 64],
                                      xrow[:, tt, h * 64:(h + 1) * 64],
                                      exw[:, tt, h:h + 1])
                # unweighted ex (cast to bf16) -> denominator cols 256:260
                nc.vector.tensor_copy(xrow[:, :, 256:260], ex[:])

                for tt in range(TPC):
                    t = ci * TPC + tt
                    if t >= TILES:
                        continue
                    b = t // TPB
                    k = t % TPB
                    if k == 0:
                        blk_ps[b] = ps2.tile([128, 260], f32, tag="blk",
                                             name="blkps")
                    ps = blk_ps[b]
                    ohx = ohp.tile([128, 128], bf16, tag="ohx")
                    nc.vector.tensor_scalar(
                        ohx[:], iota_t[:], dstoffT_t[:, t:t + 1], None,
                        Alu.is_equal)
                    nc.tensor.matmul(
                        ps[:, 0:260], ohx[:], xrow[:, tt, 0:260],
                        start=(k == 0), stop=(k == TPB - 1))
                    if k == TPB - 1:
                        if b == 0:
                            pscp = fin.tile([128, 260], f32, tag="pscp")
                            nc.scalar.copy(pscp[:], ps[:, 0:260])
                            nc.sync.dma_start(dbg_ps[:], pscp[:])
                        # finalize block b -> h, then next-layer rows
                        den = sm.tile([128, 4], f32, tag="den")
                        nc.vector.tensor_scalar_add(den[:], ps[:, 256:260], EPS)
                        rec = sm.tile([128, 4], f32, tag="rec")
                        nc.vector.reciprocal(rec[:], den[:])
                        hb = fin.tile([128, 256], f32, tag="hb")
                        for h in range(H):
                            nc.vector.scalar_tensor_tensor(
                                hb[:, h * 64:(h + 1) * 64],
                                in0=ps[:, h * 64:(h + 1) * 64],
                                scalar=rec[:, h:h + 1],
                                in1=b1b_t[:, h * 64:(h + 1) * 64],
                                op0=Alu.mult, op1=Alu.add)
                        # ELU: h = max(z,0) + exp(min(z,0)) - 1
                        zm = fin.tile([128, 256], f32, tag="zm")
                        nc.vector.tensor_scalar_min(zm[:], hb[:], 0.0)
                        ez = fin.tile([128, 256], f32, tag="ez")
                        nc.scalar.activation(ez[:], zm[:], Act.Exp)
                        nc.vector.scalar_tensor_tensor(
                            hb[:], in0=hb[:], scalar=0.0, in1=ez[:],
                            op0=Alu.max, op1=Alu.add)
                        nc.vector.tensor_scalar_add(hb[:], hb[:], -1.0)
                        if b == 0:
                            nc.sync.dma_start(dbg_hb[:], hb[:])
                        # transpose h (2 x 128x128), cast to bf16, project
                        hTs = []
                        for half in range(2):
                            pst = ps1.tile([128, 128], f32, tag="pst")
                            nc.tensor.transpose(
                                pst[:], hb[:, half * 128:(half + 1) * 128],
                                ident_t[:])
                            hT = fin.tile([128, 128], bf16, tag=f"hT{half}")
                            nc.vector.tensor_copy(hT[:], pst[:])
                            hTs.append(hT)
                        psmu = ps1.tile([128, 130], f32, tag="psmu")
                        pslv = ps1.tile([128, 130], f32, tag="pslv")
                        for kk in range(2):
                            nc.tensor.matmul(psmu[:], hTs[kk][:], wmue_t[kk][:],
                                             start=(kk == 0), stop=(kk == 1))
                            nc.tensor.matmul(pslv[:], hTs[kk][:], wlve_t[kk][:],
                                             start=(kk == 0), stop=(kk == 1))
                        xr2 = fin.tile([128, 260], bf16, tag="xr2")
                        nc.scalar.copy(xr2[:, 0:128], psmu[:, 0:128])
                        nc.scalar.copy(xr2[:, 128:256], pslv[:, 0:128])
                        ssv = xr2[:, 256:260].bitcast(f32)
                        nc.vector.tensor_copy(ssv[:, 0:1], psmu[:, 128:129])
                        nc.vector.tensor_copy(ssv[:, 1:2], pslv[:, 128:129])
                        nc.sync.dma_start(
                            agin[b * 128:(b + 1) * 128, 0:260], xr2[:])
                        sd2 = sm.tile([128, 2], f32, tag="sd2")
                        nc.vector.tensor_copy(sd2[:, 0:1], psmu[:, 129:130])
                        nc.vector.tensor_copy(sd2[:, 1:2], pslv[:, 129:130])
                        nc.sync.dma_start(
                            EXTD[b * 128:(b + 1) * 128, 4:6], sd2[:])
                        del blk_ps[b]

            nc.sync.dma_start(dbg_xpt[:], XPT[0:128, 0:264])
            nc.sync.dma_start(dbg_extd[:], EXTD[0:128, 0:6])
            nc.sync.dma_start(dbg_agin[:], agin[0:128, 0:260])

            # ---- exchange layer-2/3 message rows ----
            if upto in ("AG", "full"):
                nc.gpsimd.collective_compute(
                    "AllGather", mybir.AluOpType.bypass,
                    replica_groups=[list(range(NC))],
                    ins=[agin.opt()], outs=[agout.opt()])

            # ---- L2/3 edge phase (mu and lv share gathers) ----
            blk2 = {}
            for ci in range(NCHUNK if upto == "full" else 0):
                xrow = gx.tile([128, TPC, X2W], bf16, tag="xrow2")
                nc.gpsimd.dma_gather(
                    xrow[:], agout[:], src2_t[:, ci * 64:(ci + 1) * 64],
                    CHUNK, CHUNK, X2W)
                extd = ge.tile([128, TPC, EXW_], f32, tag="extd2")
                nc.gpsimd.dma_gather(
                    extd[:], EXTD[:], dstl_t[:, ci * 64:(ci + 1) * 64],
                    CHUNK, CHUNK, EXW_)
                z = sm.tile([128, TPC, 2], f32, tag="z2")
                nc.vector.tensor_tensor(z[:], xrow[:, :, 256:260].bitcast(f32),
                                        extd[:, :, 4:6], op=Alu.add)
                nc.vector.scalar_tensor_tensor(
                    z[:], in0=z[:], scalar=NEG, in1=z[:],
                    op0=Alu.mult, op1=Alu.max)
                ex = sm.tile([128, TPC, 2], f32, tag="ex2")
                nc.scalar.activation(ex[:], z[:], Act.Exp)
                exw = sm.tile([128, TPC, 2], f32, tag="exw2")
                wb = wT_t[:, ci * TPC:(ci + 1) * TPC]
                nc.vector.tensor_tensor(
                    exw[:], ex[:],
                    wb.rearrange("p (t o) -> p t o", o=1).to_broadcast(
                        [128, TPC, 2]), op=Alu.mult)
                exwb = sm.tile([128, TPC, 2], bf16, tag="exwb2")
                nc.vector.tensor_copy(exwb[:], exw[:])
                # scale: mu half on DVE (bcast), lv half on ACT per tile
                nc.vector.tensor_tensor(
                    xrow[:, :, 0:128], xrow[:, :, 0:128],
                    exwb[:, :, 0:1].to_broadcast([128, TPC, 128]), op=Alu.mult)
                for tt in range(TPC):
                    nc.scalar.mul(xrow[:, tt, 128:256],
                                  xrow[:, tt, 128:256], exw[:, tt, 1:2])
                nc.vector.tensor_copy(xrow[:, :, 256:258], ex[:])

                for tt in range(TPC):
                    t = ci * TPC + tt
                    if t >= TILES:
                        continue
                    b = t // TPB
                    k = t % TPB
                    if k == 0:
                        blk2[b] = ps2.tile([128, 258], f32, tag="blk",
                                           name="blk2ps")
                    ps2t = blk2[b]
                    ohx = ohp.tile([128, 128], bf16, tag="ohx")
                    nc.vector.tensor_scalar(
                        ohx[:], iota_t[:], dstoffT_t[:, t:t + 1], None,
                        Alu.is_equal)
                    nc.tensor.matmul(
                        ps2t[:, 0:258], ohx[:], xrow[:, tt, 0:258],
                        start=(k == 0), stop=(k == TPB - 1))
                    if k == TPB - 1:
                        for li, (bias_t, outdr) in enumerate(
                                ((bmub_t, mu_out), (blvb_t, lv_out))):
                            den = sm.tile([128, 1], f32, tag="den2")
                            nc.vector.tensor_scalar_add(
                                den[:], ps2t[:, 256 + li:257 + li], EPS)
                            rec = sm.tile([128, 1], f32, tag="rec2")
                            nc.vector.reciprocal(rec[:], den[:])
                            ob = fin.tile([128, 128], f32, tag="ob")
                            nc.vector.scalar_tensor_tensor(
                                ob[:], in0=ps2t[:, li * 128:(li + 1) * 128],
                                scalar=rec[:, 0:1],
                                in1=bias_t[:], op0=Alu.mult, op1=Alu.add)
                            nc.sync.dma_start(
                                outdr[b * 128:(b + 1) * 128, :], ob[:])
                        del blk2[b]

    nc.compile()
    return nc


def _prep_inputs(x, edge_index, edge_weight, W1, att1, b1, Wmu, attmu, bmu,
                 Wlv, attlv, blv):
    import ml_dtypes
    bf = ml_dtypes.bfloat16

    src = np.asarray(edge_index[0], np.int64)
    dst = np.asarray(edge_index[1], np.int64)
    w = np.asarray(edge_weight, np.float32)
    x = np.asarray(x, np.float32)

    # fused weights
    att1 = np.asarray(att1, np.float32)          # [H, 2*C1]
    W1 = np.asarray(W1, np.float32)
    Wss1 = np.zeros((FIN, H), np.float32)
    Wsd1 = np.zeros((FIN, H), np.float32)
    for h in range(H):
        Wss1[:, h] = W1[:, h * C1:(h + 1) * C1] @ att1[h, C1:]
        Wsd1[:, h] = W1[:, h * C1:(h + 1) * C1] @ att1[h, :C1]
    wse = np.concatenate([Wss1, Wsd1], axis=1)               # [512, 8]

    attmu = np.asarray(attmu, np.float32).reshape(-1)        # [2*LAT]
    attlv = np.asarray(attlv, np.float32).reshape(-1)
    Wmu = np.asarray(Wmu, np.float32)
    Wlv = np.asarray(Wlv, np.float32)
    wmue = np.concatenate(
        [Wmu, (Wmu @ attmu[LAT:])[:, None], (Wmu @ attmu[:LAT])[:, None]],
        axis=1).astype(bf)
    wlve = np.concatenate(
        [Wlv, (Wlv @ attlv[LAT:])[:, None], (Wlv @ attlv[:LAT])[:, None]],
        axis=1).astype(bf)

    xT = np.zeros((FIN, NPADA), np.float32)
    xT[:, :N] = x.T
    b1b = np.tile(np.asarray(b1, np.float32)[None, :], (128, 1))
    bmub = np.tile(np.asarray(bmu, np.float32)[None, :], (128, 1))
    blvb = np.tile(np.asarray(blv, np.float32)[None, :], (128, 1))
    iota = np.tile(np.arange(128, dtype=np.float32)[None, :],
                   (128, 1)).astype(bf)
    ident = np.eye(128, dtype=np.float32)

    # sort edges by dst, bucket per core, pad per 128-node block to TPB tiles
    order = np.argsort(dst, kind="stable")
    ssrc, sdst, sw = src[order], dst[order], w[order]
    core_of = sdst // NOWN
    in_maps = []
    for c in range(NC):
        m = core_of == c
        cs, cd, cw = ssrc[m], sdst[m] - c * NOWN, sw[m]
        blk = cd // 128
        e_src = np.zeros(EPAD, np.int64)
        e_dstloc = np.zeros(EPAD, np.int64)
        e_dstoff = np.full(EPAD, -1.0, np.float32)
        e_w = np.zeros(EPAD, np.float32)
        for b in range(BLOCKS):
            bm = blk == b
            nbe = int(bm.sum())
            if nbe > TPB * 128:
                raise RuntimeError(f"block overflow core {c} block {b}: {nbe}")
            o = b * TPB * 128
            e_src[o:o + nbe] = cs[bm]
            e_dstloc[o:o + nbe] = cd[bm]
            e_dstoff[o:o + nbe] = (cd[bm] - b * 128).astype(np.float32)
            e_w[o:o + nbe] = cw[bm]
        own = e_src // NOWN
        e_src2 = own * NLOC + (e_src - own * NOWN)
        xTown = np.zeros((FIN, NLOC), np.float32)
        xTown[:, :NOWN] = x.T[:, c * NOWN:(c + 1) * NOWN]
        in_maps.append({
            "xT": xT, "xTown": xTown, "w1b": W1.astype(bf), "wse": wse,
            "wsd_own": Wsd1, "wmue": wmue, "wlve": wlve, "b1b": b1b,
            "bmub": bmub, "blvb": blvb, "iota": iota, "ident": ident,
            "srcg": _wrap_idxs(e_src), "src2": _wrap_idxs(e_src2),
            "dstl": _wrap_idxs(e_dstloc),
            "dstoffT": _colmajor(e_dstoff),
            "wT": _colmajor(e_w),
        })
    return in_maps


def kernel(x, edge_index, edge_weight, W1, att1, b1, Wmu, attmu, bmu,
           Wlv, attlv, blv):
    from concourse.bass_utils import run_bass_kernel_spmd

    if "nc" not in _cache:
        _cache["nc"] = _build_module()
    nc = _cache["nc"]
    in_maps = _prep_inputs(x, edge_index, edge_weight, W1, att1, b1,
                           Wmu, attmu, bmu, Wlv, attlv, blv)
    r = run_bass_kernel_spmd(nc, in_maps, list(range(NC)))
    mu = np.zeros((N, LAT), np.float32)
    lv = np.zeros((N, LAT), np.float32)
    for c in range(NC):
        mu[c * NOWN:(c + 1) * NOWN] = r.results[c]["mu_out"][:NOWN]
        lv[c * NOWN:(c + 1) * NOWN] = r.results[c]["lv_out"][:NOWN]
    return (mu, lv)
